# revision 9
# baseline (speedup 1.0000x reference)
"""Trainium2 Bass kernel for the attention-scoring module:

    q = query @ Wq.T + bq                               # (B, D)
    ref[b,d,k] = sum_e enc[k,b,e] * Wref[d,e] + bref[d]
    u[b,k] = sum_d v[d] * tanh(ref[b,d,k] + q[b,d])
    out = 10 * tanh(u)                                  # (B, K)

Data-parallel over batch: core c owns b in [32c, 32c+32).

Per-core dataflow (all big tensors bf16, f32 accumulation):
  - host pre-transposes enc to (E, b*K+k) so the contraction dim E lands
    on SBUF partitions with dense DMA.
  - main matmuls: psum[d(128), n(512)] += WrefT_chunk.T @ encT_chunk
  - bias (bref+bq+q_raw[b])[d] is per-partition in this layout -> folded
    into the ScalarE tanh activation for free.
  - the v-weighted d-reduction is a second-level matmul with stationary
    v (128,1): strips (1, 512) for the four k-blocks of one b land at
    partitions {0,32,64,96} of one PSUM bank via tile_position col
    groups (bank pre-zeroed, accumulation via start=False).
  - final 10*tanh(u) runs on the whole strip window (junk rows are
    free); the per-b output DMA plucks rows {0,32,64,96} with a
    stepped-partition access pattern.
"""

import os
import sys

import numpy as np

for _p in ("/opt/trn_rl_repo", "/opt/pypackages"):
    if _p not in sys.path:
        sys.path.append(_p)

import ml_dtypes

E = 256
D = 256
K = 2048
B = 256
NCORES = 8
BL = B // NCORES          # 32 batch rows per core
N = BL * K                # 65536 flattened (b, k) per core
SLAB_B = 4                # b-rows per enc DMA slab
SLAB_N = SLAB_B * K       # 8192
C_CLIP = 10.0

_compiled = None
last_exec_time_ns = None
last_results = None


def _build():
    from concourse import bacc, bass, tile

    mybir = bass.mybir
    dt = mybir.dt
    AF = mybir.ActivationFunctionType

    nc = bacc.Bacc("TRN2", target_bir_lowering=False, debug=False,
                   num_devices=NCORES)

    enc_t = nc.declare_dram_parameter("enc_t", [E, N], dt.bfloat16, isOutput=False)
    wref_t = nc.declare_dram_parameter("wref_t", [4 * 128, 128], dt.bfloat16, isOutput=False)
    wq_t = nc.declare_dram_parameter("wq_t", [4 * 128, 128], dt.float32, isOutput=False)
    query_t = nc.declare_dram_parameter("query_t", [E, BL], dt.float32, isOutput=False)
    cbias_t = nc.declare_dram_parameter("cbias_t", [E, 1], dt.float32, isOutput=False)
    v_t = nc.declare_dram_parameter("v_t", [E, 1], dt.bfloat16, isOutput=False)
    out_p = nc.declare_dram_parameter("out", [BL, K], dt.float32, isOutput=True)

    with tile.TileContext(nc) as tc:
        with (
            tc.tile_pool(name="const", bufs=1) as constp,
            tc.tile_pool(name="enc", bufs=2) as encp,
            tc.tile_pool(name="tt", bufs=4) as tp,
            tc.tile_pool(name="tail", bufs=2) as tailp,
            tc.tile_pool(name="psum_m", bufs=3, space="PSUM") as pmp,
            tc.tile_pool(name="psum_s", bufs=2, space="PSUM") as psp,
        ):
            # ---- constants ----
            wref_sb = constp.tile([128, 512], dt.bfloat16)   # [:, (ec*2+dc)*128 + d]
            wq_sb = constp.tile([128, 512], dt.float32)
            query_sb = constp.tile([128, 2 * BL], dt.float32)  # [:, ec*32 + b]
            cbias_sb = constp.tile([128, 2], dt.float32)
            v_sb = constp.tile([128, 2], dt.bfloat16)
            bias_sb = constp.tile([128, 2 * BL], dt.float32)   # [:, dc*32 + b]

            for c in range(4):
                nc.sync.dma_start(wref_sb[:, c * 128:(c + 1) * 128],
                                  wref_t[c * 128:(c + 1) * 128, :])
                nc.sync.dma_start(wq_sb[:, c * 128:(c + 1) * 128],
                                  wq_t[c * 128:(c + 1) * 128, :])
            for ec in range(2):
                nc.sync.dma_start(query_sb[:, ec * BL:(ec + 1) * BL],
                                  query_t[ec * 128:(ec + 1) * 128, :])
            for dc in range(2):
                nc.sync.dma_start(cbias_sb[:, dc:dc + 1],
                                  cbias_t[dc * 128:(dc + 1) * 128, :])
                nc.sync.dma_start(v_sb[:, dc:dc + 1],
                                  v_t[dc * 128:(dc + 1) * 128, :])

            # ---- q_rawT = (query @ Wq.T).T per d-chunk, + (bref + bq) ----
            for dc in range(2):
                qps = psp.tile([128, BL], dt.float32, tag="st")
                for ec in range(2):
                    nc.tensor.matmul(
                        qps[:],
                        wq_sb[:, (ec * 2 + dc) * 128:(ec * 2 + dc + 1) * 128],
                        query_sb[:, ec * BL:(ec + 1) * BL],
                        start=(ec == 0), stop=(ec == 1),
                    )
                nc.vector.tensor_scalar_add(bias_sb[:, dc * BL:(dc + 1) * BL],
                                            qps[:], cbias_sb[:, dc:dc + 1])

            # ---- main loop ----
            for s in range(N // SLAB_N):            # 8 slabs of 4 b-rows
                enc_sl = []
                for ec in range(2):
                    esl = encp.tile([128, SLAB_N], dt.bfloat16, tag=f"enc{ec}")
                    nc.sync.dma_start(
                        esl[:],
                        enc_t[ec * 128:(ec + 1) * 128, s * SLAB_N:(s + 1) * SLAB_N])
                    enc_sl.append(esl)

                for b_in in range(SLAB_B):
                    b = SLAB_B * s + b_in
                    st4 = psp.tile([128, 512], dt.float32, tag="st")
                    nc.vector.memset(st4[:], 0.0)
                    for kp in range(2):             # two 1024-wide n groups
                        tts = []
                        for dc in range(2):
                            psd = pmp.tile([128, 1024], dt.float32, tag="psd")
                            for kb in range(2):
                                nseg = b_in * K + kp * 1024 + kb * 512
                                for ec in range(2):
                                    nc.tensor.matmul(
                                        psd[:, kb * 512:(kb + 1) * 512],
                                        wref_sb[:, (ec * 2 + dc) * 128:(ec * 2 + dc + 1) * 128],
                                        enc_sl[ec][:, nseg:nseg + 512],
                                        start=(ec == 0), stop=(ec == 1),
                                    )
                            ttile = tp.tile([128, 1024], dt.bfloat16, tag="tt")
                            nc.scalar.activation(
                                ttile[:], psd[:], AF.Tanh,
                                bias=bias_sb[:, dc * BL + b:dc * BL + b + 1],
                                scale=1.0)
                            tts.append(ttile)
                        for kb in range(2):
                            jj = kp * 2 + kb
                            for dc in range(2):
                                nc.tensor.matmul(
                                    st4[32 * jj:32 * jj + 1, :],
                                    v_sb[:, dc:dc + 1],
                                    tts[dc][:, kb * 512:(kb + 1) * 512],
                                    start=False, stop=(dc == 1),
                                    skip_group_check=True,
                                    tile_position=(0, 32 * jj),
                                )
                    # out[b, :] = 10 * tanh(strips); junk rows are free
                    t5 = tailp.tile([128, 512], dt.float32, tag="t5")
                    nc.scalar.activation(t5[:], st4[:], AF.Tanh)
                    o5 = tailp.tile([128, 512], dt.float32, tag="o5")
                    nc.vector.tensor_scalar_mul(o5[:], t5[:], C_CLIP)
                    nc.sync.dma_start(out_p[b:b + 1, :], o5[0:128:32, :])

    nc.compile()
    return nc


def _prep_inputs(encoder_output, query, Wq, bq, Wref, bref, v):
    bf16 = ml_dtypes.bfloat16
    # (K, B, E) -> (E, B, K), bf16
    enc_bf = np.asarray(encoder_output, np.float32).astype(bf16)
    encT = enc_bf.transpose(2, 1, 0)                   # (E, B, K) view

    def chunk4(w):                                     # (E, D) -> (4*128, 128)
        return np.ascontiguousarray(
            w.reshape(2, 128, 2, 128).transpose(0, 2, 1, 3).reshape(512, 128))

    wrefT = chunk4(np.asarray(Wref, np.float32).T).astype(bf16)
    wqT = chunk4(np.asarray(Wq, np.float32).T)
    cbias = (np.asarray(bref, np.float32) + np.asarray(bq, np.float32)).reshape(E, 1)
    v_col = np.asarray(v, np.float32).astype(bf16).reshape(E, 1)
    queryT = np.ascontiguousarray(np.asarray(query, np.float32).T)  # (E, B)

    in_maps = []
    for c in range(NCORES):
        enc_c = np.ascontiguousarray(encT[:, c * BL:(c + 1) * BL, :]).reshape(E, N)
        in_maps.append({
            "enc_t": enc_c,
            "wref_t": wrefT,
            "wq_t": wqT,
            "query_t": np.ascontiguousarray(queryT[:, c * BL:(c + 1) * BL]),
            "cbias_t": cbias,
            "v_t": v_col,
        })
    return in_maps


def kernel(**inputs):
    global _compiled, last_exec_time_ns, last_results
    from concourse import bass_utils

    if _compiled is None:
        _compiled = _build()
    nc = _compiled

    in_maps = _prep_inputs(**inputs)
    res = bass_utils.run_bass_kernel_spmd(nc, in_maps, core_ids=list(range(NCORES)))
    last_exec_time_ns = res.exec_time_ns
    last_results = res
    out = np.concatenate([r["out"] for r in res.results], axis=0)
    return out


# revision 12
# speedup vs baseline: 1.2075x; 1.2075x over previous
"""Trainium2 Bass kernel for the attention-scoring module:

    q = query @ Wq.T + bq                               # (B, D)
    ref[b,d,k] = sum_e enc[k,b,e] * Wref[d,e] + bref[d]
    u[b,k] = sum_d v[d] * tanh(ref[b,d,k] + q[b,d])
    out = 10 * tanh(u)                                  # (B, K)

Data-parallel over batch: core c owns b in [32c, 32c+32).

Per-core dataflow (all big tensors bf16, f32 accumulation):
  - host pre-transposes enc to (E, b*K+k) so the contraction dim E lands
    on SBUF partitions with dense DMA.
  - main matmuls: psum[d(128), n(512)] += WrefT_chunk.T @ encT_chunk
  - bias (bref+bq+q_raw[b])[d] is per-partition in this layout -> folded
    into the ScalarE tanh activation for free.
  - the v-weighted d-reduction is a second-level matmul with stationary
    v (128,1): strips (1, 512) for the four k-blocks of one b land at
    partitions {0,32,64,96} of one PSUM bank via tile_position col
    groups (bank pre-zeroed, accumulation via start=False).
  - final 10*tanh(u) runs on the whole strip window (junk rows are
    free); the per-b output DMA plucks rows {0,32,64,96} with a
    stepped-partition access pattern.
"""

import os
import sys

import numpy as np

for _p in ("/opt/trn_rl_repo", "/opt/pypackages"):
    if _p not in sys.path:
        sys.path.append(_p)

import ml_dtypes

E = 256
D = 256
K = 2048
B = 256
NCORES = 8
BL = B // NCORES          # 32 batch rows per core
N = BL * K                # 65536 flattened (b, k) per core
SLAB_B = 4                # b-rows per enc DMA slab
SLAB_N = SLAB_B * K       # 8192
C_CLIP = 10.0

_compiled = None
last_exec_time_ns = None
last_results = None


def _build():
    from concourse import bacc, bass, tile

    mybir = bass.mybir
    dt = mybir.dt
    AF = mybir.ActivationFunctionType

    nc = bacc.Bacc("TRN2", target_bir_lowering=False, debug=False,
                   num_devices=NCORES)

    enc_t = nc.declare_dram_parameter("enc_t", [E, N], dt.bfloat16, isOutput=False)
    wref_t = nc.declare_dram_parameter("wref_t", [4 * 128, 128], dt.bfloat16, isOutput=False)
    wq_t = nc.declare_dram_parameter("wq_t", [4 * 128, 128], dt.float32, isOutput=False)
    query_t = nc.declare_dram_parameter("query_t", [E, BL], dt.float32, isOutput=False)
    cbias_t = nc.declare_dram_parameter("cbias_t", [E, 1], dt.float32, isOutput=False)
    v_t = nc.declare_dram_parameter("v_t", [E, 1], dt.bfloat16, isOutput=False)
    out_p = nc.declare_dram_parameter("out", [BL, K], dt.float32, isOutput=True)

    with tile.TileContext(nc) as tc:
        with (
            tc.tile_pool(name="const", bufs=1) as constp,
            tc.tile_pool(name="enc", bufs=3) as encp,
            tc.tile_pool(name="tt", bufs=6) as tp,
            tc.tile_pool(name="tail", bufs=2) as tailp,
            tc.tile_pool(name="psum_m", bufs=3, space="PSUM") as pmp,
            tc.tile_pool(name="psum_s", bufs=2, space="PSUM") as psp,
        ):
            # ---- enc slab loading (slab 0 first, split per b-row so the
            # first matmuls start as early as possible) ----
            def load_slab(s, split):
                tiles = []
                for ec in range(2):
                    esl = encp.tile([128, SLAB_N], dt.bfloat16, tag=f"enc{ec}")
                    if split:
                        for q in range(SLAB_B):
                            nc.sync.dma_start(
                                esl[:, q * K:(q + 1) * K],
                                enc_t[ec * 128:(ec + 1) * 128,
                                      s * SLAB_N + q * K:s * SLAB_N + (q + 1) * K])
                    else:
                        nc.sync.dma_start(
                            esl[:],
                            enc_t[ec * 128:(ec + 1) * 128,
                                  s * SLAB_N:(s + 1) * SLAB_N])
                    tiles.append(esl)
                return tiles

            cur_slab = load_slab(0, split=True)

            # ---- constants ----
            wref_sb = constp.tile([128, 512], dt.bfloat16)   # [:, (ec*2+dc)*128 + d]
            wq_sb = constp.tile([128, 512], dt.float32)
            query_sb = constp.tile([128, 2 * BL], dt.float32)  # [:, ec*32 + b]
            cbias_sb = constp.tile([128, 2], dt.float32)
            v_sb = constp.tile([128, 2], dt.bfloat16)
            bias_sb = constp.tile([128, 2 * BL], dt.float32)   # [:, dc*32 + b]

            for c in range(4):
                nc.sync.dma_start(wref_sb[:, c * 128:(c + 1) * 128],
                                  wref_t[c * 128:(c + 1) * 128, :])
                nc.sync.dma_start(wq_sb[:, c * 128:(c + 1) * 128],
                                  wq_t[c * 128:(c + 1) * 128, :])
            for ec in range(2):
                nc.sync.dma_start(query_sb[:, ec * BL:(ec + 1) * BL],
                                  query_t[ec * 128:(ec + 1) * 128, :])
            for dc in range(2):
                nc.sync.dma_start(cbias_sb[:, dc:dc + 1],
                                  cbias_t[dc * 128:(dc + 1) * 128, :])
                nc.sync.dma_start(v_sb[:, dc:dc + 1],
                                  v_t[dc * 128:(dc + 1) * 128, :])

            # ---- q_rawT = (query @ Wq.T).T per d-chunk, + (bref + bq) ----
            for dc in range(2):
                qps = psp.tile([128, BL], dt.float32, tag="st")
                for ec in range(2):
                    nc.tensor.matmul(
                        qps[:],
                        wq_sb[:, (ec * 2 + dc) * 128:(ec * 2 + dc + 1) * 128],
                        query_sb[:, ec * BL:(ec + 1) * BL],
                        start=(ec == 0), stop=(ec == 1),
                    )
                nc.vector.tensor_scalar_add(bias_sb[:, dc * BL:(dc + 1) * BL],
                                            qps[:], cbias_sb[:, dc:dc + 1])

            # ---- main loop, v-matmuls software-pipelined one round behind
            # the main matmuls so they never stall TensorE on ScalarE ----
            def emit_epilogue(st4, tts, b, kp):
                for kb in range(2):
                    jj = kp * 2 + kb
                    for dc in range(2):
                        nc.tensor.matmul(
                            st4[32 * jj:32 * jj + 1, :],
                            v_sb[:, dc:dc + 1],
                            tts[dc][:, kb * 512:(kb + 1) * 512],
                            start=False, stop=(dc == 1),
                            skip_group_check=True,
                            tile_position=(0, 32 * jj),
                        )
                if kp == 1:
                    # out[b, :] = 10 * tanh(strips); junk rows are free
                    t5 = tailp.tile([128, 512], dt.float32, tag="t5")
                    nc.scalar.activation(t5[:], st4[:], AF.Tanh)
                    o5 = tailp.tile([128, 512], dt.float32, tag="o5")
                    nc.vector.tensor_scalar_mul(o5[:], t5[:], C_CLIP)
                    nc.sync.dma_start(out_p[b:b + 1, :], o5[0:128:32, :])

            pend = None
            for s in range(N // SLAB_N):            # 8 slabs of 4 b-rows
                nxt_slab = load_slab(s + 1, split=False) if s + 1 < N // SLAB_N else None
                for b_in in range(SLAB_B):
                    b = SLAB_B * s + b_in
                    st4 = psp.tile([128, 512], dt.float32, tag="st")
                    nc.vector.memset(st4[:], 0.0)
                    for kp in range(2):             # two 1024-wide n groups
                        tts = []
                        for dc in range(2):
                            psd = pmp.tile([128, 1024], dt.float32, tag="psd")
                            for kb in range(2):
                                nseg = b_in * K + kp * 1024 + kb * 512
                                for ec in range(2):
                                    nc.tensor.matmul(
                                        psd[:, kb * 512:(kb + 1) * 512],
                                        wref_sb[:, (ec * 2 + dc) * 128:(ec * 2 + dc + 1) * 128],
                                        cur_slab[ec][:, nseg:nseg + 512],
                                        start=(ec == 0), stop=(ec == 1),
                                    )
                            ttile = tp.tile([128, 1024], dt.bfloat16, tag="tt")
                            nc.scalar.activation(
                                ttile[:], psd[:], AF.Tanh,
                                bias=bias_sb[:, dc * BL + b:dc * BL + b + 1],
                                scale=1.0)
                            tts.append(ttile)
                        if pend is not None:
                            emit_epilogue(*pend)
                        pend = (st4, tts, b, kp)
                cur_slab = nxt_slab
            emit_epilogue(*pend)

    nc.compile()
    return nc


def _prep_inputs(encoder_output, query, Wq, bq, Wref, bref, v):
    bf16 = ml_dtypes.bfloat16
    # (K, B, E) -> (E, B, K), bf16
    enc_bf = np.asarray(encoder_output, np.float32).astype(bf16)
    encT = enc_bf.transpose(2, 1, 0)                   # (E, B, K) view

    def chunk4(w):                                     # (E, D) -> (4*128, 128)
        return np.ascontiguousarray(
            w.reshape(2, 128, 2, 128).transpose(0, 2, 1, 3).reshape(512, 128))

    wrefT = chunk4(np.asarray(Wref, np.float32).T).astype(bf16)
    wqT = chunk4(np.asarray(Wq, np.float32).T)
    cbias = (np.asarray(bref, np.float32) + np.asarray(bq, np.float32)).reshape(E, 1)
    v_col = np.asarray(v, np.float32).astype(bf16).reshape(E, 1)
    queryT = np.ascontiguousarray(np.asarray(query, np.float32).T)  # (E, B)

    in_maps = []
    for c in range(NCORES):
        enc_c = np.ascontiguousarray(encT[:, c * BL:(c + 1) * BL, :]).reshape(E, N)
        in_maps.append({
            "enc_t": enc_c,
            "wref_t": wrefT,
            "wq_t": wqT,
            "query_t": np.ascontiguousarray(queryT[:, c * BL:(c + 1) * BL]),
            "cbias_t": cbias,
            "v_t": v_col,
        })
    return in_maps


def kernel(**inputs):
    global _compiled, last_exec_time_ns, last_results
    from concourse import bass_utils

    if _compiled is None:
        _compiled = _build()
    nc = _compiled

    in_maps = _prep_inputs(**inputs)
    res = bass_utils.run_bass_kernel_spmd(nc, in_maps, core_ids=list(range(NCORES)))
    last_exec_time_ns = res.exec_time_ns
    last_results = res
    out = np.concatenate([r["out"] for r in res.results], axis=0)
    return out
